# revision 4
# baseline (speedup 1.0000x reference)
"""Bass/Trainium2 kernel for nn_Attention_54099408060779.

out[b] = softmax(q[b] @ k[b].T) @ v[b]   (no scaling, no mask)
B=8, S=4096, D=64, fp32 I/O.

Sharding: pure data parallel — batch b runs on NeuronCore b.

Per-core algorithm (flash-attention style, never materializes [S, S] in DRAM):
  - q, k, v are loaded as flat contiguous [128, 32, 64] f32 tiles (rows
    32*p .. 32*p+31 on partition p) so each load is a cheap 128-descriptor
    DMA, spread across the three DMA queues.  This permutes the row
    enumeration to s = 32*p + x; the permutation is carried consistently
    through the whole pipeline and undoes itself on the flat output store.
  - GPSIMD casts q,k to fp16 (dup / even-odd interleaved layouts) and v to
    bf16 (+ ones column) with strided tensor_copies — no ACT/DVE/PE cost.
  - The XBAR DMA-transpose produces qT ([dup*64+d, q], fp16, d duplicated
    in both partition halves) and kT2 ([par*64+d, key], even/odd key-tiles
    interleaved) for row-tiled mm1.
  - For each 512-wide q block, for each pair of 128-key tiles:
      sT[keys, q] = kT_pair.T @ qT_block   (TensorE, fp16, row-tiled 2x)
      p = exp(sT)  split across two engines:
        ScalarE: exact exp activation -> bf16
        VectorE: Schraudolph bit-trick -> int16 bits == bf16(exp(s)):
                 i16 = rint(s * 128*log2e + (128*127 - 5.5)), ~3.3% max err
      oT[65, q] += va.T @ p                (TensorE, bf16, ones column
                                            gives the softmax row-sums l)
  - PE-transpose oT back to [q, 65], multiply by 1/l into out_sb, and store
    the whole output with one flat DMA.

exp needs no max-subtraction: scores ~ N(0, 64), |s| < ~70 and fp32/bf16
range comfortably holds exp(70); the Schraudolph i16 stays in [3.7k, 29k].
"""

import sys

if "/opt/trn_rl_repo" not in sys.path:
    sys.path.insert(0, "/opt/trn_rl_repo")

import numpy as np

import concourse.bacc as bacc
import concourse.tile as tile
from concourse import mybir
from concourse.bass_utils import run_bass_kernel_spmd
from concourse.masks import make_identity

B, S, D = 8, 4096, 64
P = 128                # SBUF partitions / k-tile height
NKT = S // P           # 32 k-tiles (x-blocks)
NPAIR = NKT // 2       # 16 row-tiled pairs
QB = 512               # q-block width (matmul moving free dim)
NQB = S // QB          # 8 q-blocks
ROWS = S // P          # 32 rows per partition in the flat layout

BF16 = mybir.dt.bfloat16
F16 = mybir.dt.float16
F32 = mybir.dt.float32
F32R = mybir.dt.float32r
I16 = mybir.dt.int16
EXP = mybir.ActivationFunctionType.Exp

# Schraudolph constants: i16 bits of bf16(exp(s)) ~= rint(s*SCH_A + SCH_B)
SCH_A = float(128.0 * np.log2(np.e))
SCH_B = float(128.0 * 127.0 - 5.5)

# exp-engine split: ScalarE takes even pairs (exact exp), VectorE
# (Schraudolph) the odd ones.
def _use_act(qb, i):
    return i % 2 == 0

import os as _os
DEFAULTS = dict(
    sps=int(_os.environ.get("K_SPS", "3")),
    ops=int(_os.environ.get("K_OPS", "2")),
    eps=int(_os.environ.get("K_EPS", "4")),
)

_CACHE: dict = {}


def _build(reps: int = 1, **over):
    cfg = dict(DEFAULTS); cfg.update(over)
    nc = bacc.Bacc(None, target_bir_lowering=False)
    q = nc.dram_tensor("q", [S, D], F32, kind="ExternalInput")
    k = nc.dram_tensor("k", [S, D], F32, kind="ExternalInput")
    v = nc.dram_tensor("v", [S, D], F32, kind="ExternalInput")
    out = nc.dram_tensor("out", [S, D], F32, kind="ExternalOutput")

    with tile.TileContext(nc) as tc:
        with (
            tc.tile_pool(name="consts", bufs=1) as consts,
            tc.tile_pool(name="big", bufs=2) as big,
            tc.tile_pool(name="s_ps", bufs=cfg["sps"], space="PSUM") as s_ps,
            tc.tile_pool(name="o_ps", bufs=cfg["ops"], space="PSUM") as o_ps,
            tc.tile_pool(name="pp", bufs=5) as pp,
            tc.tile_pool(name="ep", bufs=cfg["eps"]) as ep,
        ):
            for _rep in range(reps):
                _kernel_body(nc, tc, q, k, v, out, big, s_ps, o_ps, pp, ep)

    nc.finalize()
    return nc


def _kernel_body(nc, tc, q, k, v, out, big, s_ps, o_ps, pp, ep):
    # flat f32 stages (row 32*p + x lives at [p, x, :])
    qflat = big.tile([P, ROWS, D], F32, name="qflat")
    kflat = big.tile([P, ROWS, D], F32, name="kflat")
    vflat = big.tile([P, ROWS, D], F32, name="vflat")
    # fp16 interleaved stages for the XBAR
    q16i = big.tile([P, ROWS, 2, D], F16, name="q16i")     # (x, dup, d)
    k16i = big.tile([P, NPAIR, 2, D], F16, name="k16i")    # (pair, parity, d)
    qT = big.tile([P, S], F16, name="qT")                  # [dup*64+d, x*128+p]
    kT2 = big.tile([P, NPAIR * P], F16, name="kT2")        # [par*64+d, b*128+p]
    va = big.tile([P, NKT, D + 1], BF16, name="va")
    out_sb = big.tile([P, ROWS, D], F32, name="out_sb")

    # flat loads: 2 chunks each, spread across the three DMA queues
    qsrc = q.rearrange("(p x) d -> p x d", p=P)
    ksrc = k.rearrange("(p x) d -> p x d", p=P)
    vsrc = v.rearrange("(p x) d -> p x d", p=P)
    kview = kflat.rearrange("p (b t) d -> p b t d", t=2)
    for h in range(2):
        xs = slice(h * (ROWS // 2), (h + 1) * (ROWS // 2))
        bs = slice(h * (NPAIR // 2), (h + 1) * (NPAIR // 2))
        nc.scalar.dma_start(out=qflat[:, xs, :], in_=qsrc[:, xs, :])
        nc.sync.dma_start(out=kflat[:, xs, :], in_=ksrc[:, xs, :])
        nc.gpsimd.dma_start(out=vflat[:, xs, :], in_=vsrc[:, xs, :])
        # casts + interleaves on GPSIMD (SBUF->SBUF, engine otherwise idle)
        # k16i's (pair, parity, d) order equals kflat's (x, d) order, so the
        # cast is one contiguous copy
        nc.gpsimd.tensor_copy(
            out=k16i[:, bs, :, :], in_=kview[:, bs, :, :]
        )
        for dup in range(2):
            nc.gpsimd.tensor_copy(out=q16i[:, xs, dup, :], in_=qflat[:, xs, :])
        nc.gpsimd.tensor_copy(out=va[:, xs, 0:D], in_=vflat[:, xs, :])
        # XBAR transposes (HWDGE on SP queue)
        for b in range(bs.start, bs.stop):
            nc.sync.dma_start(
                out=kT2[:, b * P : (b + 1) * P], in_=k16i[:, b, :, :], transpose=True
            )
        for x in range(xs.start, xs.stop):
            nc.sync.dma_start(
                out=qT[:, x * P : (x + 1) * P], in_=q16i[:, x, :, :], transpose=True
            )
    nc.vector.memset(va[:, :, D : D + 1], 1.0)

    def emit_mm2(oTs, i, pg):
        # Row-tiled: the K=128 key contraction splits into two K=64 strips at
        # tile_position (0,0)/(64,0) so the strips' matmuls stream through
        # disjoint array halves concurrently and each strip's LDWEIGHTS hides
        # under the other strip's in-flight matmul (the full-K version pays an
        # exposed ~230ns weight-swap per matmul, ~3x slowdown on this stream).
        # The accumulator bank is keyed by STRIP (oTs[r]): a bank is only ever
        # written by its own strip, so same-bank PE drains are strictly serial
        # (concurrent same-bank drains fault the device) while the two
        # strips/banks run concurrently.  oT = oTs[0] + oTs[1] still holds,
        # just partitioned by key-half instead of kt-parity.
        for half in range(2):
            kt = 2 * i + half
            for r in range(2):
                nc.tensor.matmul(
                    oTs[r],
                    lhsT=va[r * D : (r + 1) * D, kt, :],
                    rhs=pg[r * D : (r + 1) * D, half * QB : (half + 1) * QB],
                    start=(i == 0 and half == 0),
                    stop=(i == NPAIR - 1 and half == 1),
                    tile_position=(r * D, 0),
                )

    def epilogue(qb, oTs):
        # combine the two mm2 accumulator banks into a bf16 staging tile
        # (ACT stages bank B to SBUF since DVE reads only one PSUM operand),
        # XBAR-transpose back to [q, 65], normalize into out_sb
        tmpB = ep.tile([P, QB], F32, name="tmpB")
        nc.scalar.copy(out=tmpB[0 : D + 1, :], in_=oTs[1])
        oT_sb16 = ep.tile([P, QB], BF16, name="oT_sb16")
        # rows 65:80 are read by the padded XBAR transpose; keep them defined
        # (base partition must be 32-aligned; the add overwrites row 64)
        nc.vector.memset(oT_sb16[D : 96, :], 0.0)
        nc.vector.tensor_add(
            out=oT_sb16[0 : D + 1, :], in0=oTs[0], in1=tmpB[0 : D + 1, :]
        )
        for j in range(QB // P):
            x = 4 * qb + j
            tp16 = ep.tile([P, 80], BF16, name="tp16")
            nc.sync.dma_start(
                out=tp16, in_=oT_sb16[0:80, j * P : (j + 1) * P], transpose=True
            )
            rec = ep.tile([P, 1], F32, name="rec")
            nc.vector.reciprocal(rec, tp16[:, D : D + 1])
            nc.vector.tensor_scalar_mul(
                out_sb[:, x, :], tp16[:, 0:D], rec
            )

    # 2-group software lookahead: emit mm1(i) + exp(i), then mm2(i-2), so PE
    # always has independent mm1 work in front of it and never waits on the
    # exp of the group it is about to consume (lag 1 leaves PE stalling on
    # exp completions; measured ~16us slower).
    LAG = 2
    pend = []

    def _drain_one():
        kind, args = pend.pop(0)
        if kind == "mm2":
            emit_mm2(*args)
        else:
            epilogue(*args)

    for qb in range(NQB):
        oTs = [o_ps.tile([D + 1, QB], F32, name="oT") for _ in range(2)]
        for i in range(NPAIR):
            sg = s_ps.tile([P, 2 * QB], F32, name="sg")
            nc.tensor.matmul(
                sg[:, 0:QB],
                lhsT=kT2[0:D, i * P : (i + 1) * P],
                rhs=qT[0:D, qb * QB : (qb + 1) * QB],
                start=True,
                stop=True,
                tile_position=(0, 0),
            )
            nc.tensor.matmul(
                sg[:, QB : 2 * QB],
                lhsT=kT2[D:P, i * P : (i + 1) * P],
                rhs=qT[D:P, qb * QB : (qb + 1) * QB],
                start=True,
                stop=True,
                tile_position=(D, 0),
            )
            pg = pp.tile([P, 2 * QB], BF16, name="pg")
            if _use_act(qb, i):
                nc.scalar.activation(pg, sg, EXP)
            else:
                nc.vector.tensor_scalar(
                    out=pg.bitcast(I16),
                    in0=sg,
                    scalar1=SCH_A,
                    scalar2=SCH_B,
                    op0=mybir.AluOpType.mult,
                    op1=mybir.AluOpType.add,
                )
            pend.append(("mm2", (oTs, i, pg)))
            while len([e for e in pend if e[0] == "mm2"]) > LAG:
                _drain_one()
        pend.append(("epi", (qb, oTs)))
    while pend:
        _drain_one()

    # one flat store: out_sb[p, x, :] -> out row 32*p + x
    nc.scalar.dma_start(
        out=out.rearrange("(p x) d -> p x d", p=P), in_=out_sb
    )


def get_nc():
    if "nc" not in _CACHE:
        _CACHE["nc"] = _build()
    return _CACHE["nc"]


def kernel(q3d, k3d, v3d, _trace=False):
    q3d = np.ascontiguousarray(np.asarray(q3d, dtype=np.float32))
    k3d = np.ascontiguousarray(np.asarray(k3d, dtype=np.float32))
    v3d = np.ascontiguousarray(np.asarray(v3d, dtype=np.float32))
    assert q3d.shape == (B, S, D), q3d.shape

    nc = get_nc()
    in_maps = [{"q": q3d[b], "k": k3d[b], "v": v3d[b]} for b in range(B)]
    try:
        res = run_bass_kernel_spmd(nc, in_maps, core_ids=list(range(B)), trace=_trace)
    except Exception:
        # transient NRT/device wedges have been observed to clear on retry
        res = run_bass_kernel_spmd(nc, in_maps, core_ids=list(range(B)), trace=_trace)
    out = np.stack([res.results[b]["out"] for b in range(B)], axis=0)
    if _trace:
        return out, res
    return out


if __name__ == "__main__":
    rng = np.random.default_rng(0)
    qq = rng.standard_normal((B, S, D), dtype=np.float32)
    kk = rng.standard_normal((B, S, D), dtype=np.float32)
    vv = rng.standard_normal((B, S, D), dtype=np.float32)
    o = kernel(q3d=qq, k3d=kk, v3d=vv)
    print("kernel output:", o.shape, o.dtype)



# revision 5
# speedup vs baseline: 1.1829x; 1.1829x over previous
"""Bass/Trainium2 kernel for nn_Attention_54099408060779.

out[b] = softmax(q[b] @ k[b].T) @ v[b]   (no scaling, no mask)
B=8, S=4096, D=64, fp32 I/O.

Sharding: pure data parallel — batch b runs on NeuronCore b.

Per-core algorithm (flash-attention style, never materializes [S, S] in DRAM):
  - q, k, v are loaded as flat contiguous [128, 32, 64] f32 tiles (rows
    32*p .. 32*p+31 on partition p) so each load is a cheap 128-descriptor
    DMA, spread across the three DMA queues.  This permutes the row
    enumeration to s = 32*p + x; the permutation is carried consistently
    through the whole pipeline and undoes itself on the flat output store.
  - GPSIMD casts q,k to fp16 (dup / even-odd interleaved layouts) and v to
    bf16 (+ ones column) with strided tensor_copies — no ACT/DVE/PE cost.
  - The XBAR DMA-transpose produces qT ([dup*64+d, q], fp16, d duplicated
    in both partition halves) and kT2 ([par*64+d, key], even/odd key-tiles
    interleaved) for row-tiled mm1.
  - For each 512-wide q block, for each pair of 128-key tiles:
      sT[keys, q] = kT_pair.T @ qT_block   (TensorE, fp16, row-tiled 2x)
      p = exp(sT)  split across two engines:
        ScalarE: exact exp activation -> bf16
        VectorE: Schraudolph bit-trick -> int16 bits == bf16(exp(s)):
                 i16 = rint(s * 128*log2e + (128*127 - 5.5)), ~3.3% max err
      oT[65, q] += va.T @ p                (TensorE, bf16, ones column
                                            gives the softmax row-sums l)
  - PE-transpose oT back to [q, 65], multiply by 1/l into out_sb, and store
    the whole output with one flat DMA.

exp needs no max-subtraction: scores ~ N(0, 64), |s| < ~70 and fp32/bf16
range comfortably holds exp(70); the Schraudolph i16 stays in [3.7k, 29k].
"""

import sys

if "/opt/trn_rl_repo" not in sys.path:
    sys.path.insert(0, "/opt/trn_rl_repo")

import numpy as np

import concourse.bacc as bacc
import concourse.tile as tile
from concourse import mybir
from concourse.bass_utils import run_bass_kernel_spmd
from concourse.masks import make_identity

B, S, D = 8, 4096, 64
P = 128                # SBUF partitions / k-tile height
NKT = S // P           # 32 k-tiles (x-blocks)
NPAIR = NKT // 2       # 16 row-tiled pairs
QB = 512               # q-block width (matmul moving free dim)
NQB = S // QB          # 8 q-blocks
ROWS = S // P          # 32 rows per partition in the flat layout

BF16 = mybir.dt.bfloat16
F16 = mybir.dt.float16
F32 = mybir.dt.float32
F32R = mybir.dt.float32r
I16 = mybir.dt.int16
EXP = mybir.ActivationFunctionType.Exp

# Schraudolph constants: i16 bits of bf16(exp(s)) ~= rint(s*SCH_A + SCH_B)
SCH_A = float(128.0 * np.log2(np.e))
SCH_B = float(128.0 * 127.0 - 5.5)

# exp engine: all tiles on ScalarE (exact exp).  ACT and DVE PSUM reads
# serialize against each other on TRN2, so splitting exp across both engines
# costs the SUM of their op times; the single faster engine (ACT, 1.2 GHz)
# wins outright and frees DVE for the epilogue (A/B measured -25us vs the
# alternating split).  The Schraudolph DVE path below is kept for reference.
def _use_act(qb, i):
    return True

import os as _os
DEFAULTS = dict(
    sps=int(_os.environ.get("K_SPS", "3")),
    ops=int(_os.environ.get("K_OPS", "2")),
    eps=int(_os.environ.get("K_EPS", "4")),
)

_CACHE: dict = {}


def _build(reps: int = 1, **over):
    cfg = dict(DEFAULTS); cfg.update(over)
    nc = bacc.Bacc(None, target_bir_lowering=False)
    q = nc.dram_tensor("q", [S, D], F32, kind="ExternalInput")
    k = nc.dram_tensor("k", [S, D], F32, kind="ExternalInput")
    v = nc.dram_tensor("v", [S, D], F32, kind="ExternalInput")
    out = nc.dram_tensor("out", [S, D], F32, kind="ExternalOutput")

    with tile.TileContext(nc) as tc:
        with (
            tc.tile_pool(name="consts", bufs=1) as consts,
            tc.tile_pool(name="big", bufs=2) as big,
            tc.tile_pool(name="s_ps", bufs=cfg["sps"], space="PSUM") as s_ps,
            tc.tile_pool(name="o_ps", bufs=cfg["ops"], space="PSUM") as o_ps,
            tc.tile_pool(name="pp", bufs=5) as pp,
            tc.tile_pool(name="ep", bufs=cfg["eps"]) as ep,
        ):
            for _rep in range(reps):
                _kernel_body(nc, tc, q, k, v, out, big, s_ps, o_ps, pp, ep)

    nc.finalize()
    return nc


def _kernel_body(nc, tc, q, k, v, out, big, s_ps, o_ps, pp, ep):
    # flat f32 stages (row 32*p + x lives at [p, x, :])
    qflat = big.tile([P, ROWS, D], F32, name="qflat")
    kflat = big.tile([P, ROWS, D], F32, name="kflat")
    vflat = big.tile([P, ROWS, D], F32, name="vflat")
    # fp16 interleaved stages for the XBAR
    q16i = big.tile([P, ROWS, 2, D], F16, name="q16i")     # (x, dup, d)
    k16i = big.tile([P, NPAIR, 2, D], F16, name="k16i")    # (pair, parity, d)
    qT = big.tile([P, S], F16, name="qT")                  # [dup*64+d, x*128+p]
    kT2 = big.tile([P, NPAIR * P], F16, name="kT2")        # [par*64+d, b*128+p]
    va = big.tile([P, NKT, D + 1], BF16, name="va")
    out_sb = big.tile([P, ROWS, D], F32, name="out_sb")

    # flat loads: 2 chunks each, spread across the three DMA queues
    qsrc = q.rearrange("(p x) d -> p x d", p=P)
    ksrc = k.rearrange("(p x) d -> p x d", p=P)
    vsrc = v.rearrange("(p x) d -> p x d", p=P)
    kview = kflat.rearrange("p (b t) d -> p b t d", t=2)
    for h in range(2):
        xs = slice(h * (ROWS // 2), (h + 1) * (ROWS // 2))
        bs = slice(h * (NPAIR // 2), (h + 1) * (NPAIR // 2))
        nc.scalar.dma_start(out=qflat[:, xs, :], in_=qsrc[:, xs, :])
        nc.sync.dma_start(out=kflat[:, xs, :], in_=ksrc[:, xs, :])
        nc.gpsimd.dma_start(out=vflat[:, xs, :], in_=vsrc[:, xs, :])
        # casts + interleaves on GPSIMD (SBUF->SBUF, engine otherwise idle)
        # k16i's (pair, parity, d) order equals kflat's (x, d) order, so the
        # cast is one contiguous copy
        nc.gpsimd.tensor_copy(
            out=k16i[:, bs, :, :], in_=kview[:, bs, :, :]
        )
        for dup in range(2):
            nc.gpsimd.tensor_copy(out=q16i[:, xs, dup, :], in_=qflat[:, xs, :])
        nc.gpsimd.tensor_copy(out=va[:, xs, 0:D], in_=vflat[:, xs, :])
        # XBAR transposes (HWDGE on SP queue)
        for b in range(bs.start, bs.stop):
            nc.sync.dma_start(
                out=kT2[:, b * P : (b + 1) * P], in_=k16i[:, b, :, :], transpose=True
            )
        for x in range(xs.start, xs.stop):
            nc.sync.dma_start(
                out=qT[:, x * P : (x + 1) * P], in_=q16i[:, x, :, :], transpose=True
            )
    nc.vector.memset(va[:, :, D : D + 1], 1.0)

    def emit_mm2(oTs, i, pg):
        # Row-tiled: the K=128 key contraction splits into two K=64 strips at
        # tile_position (0,0)/(64,0) so the strips' matmuls stream through
        # disjoint array halves concurrently and each strip's LDWEIGHTS hides
        # under the other strip's in-flight matmul (the full-K version pays an
        # exposed ~230ns weight-swap per matmul, ~3x slowdown on this stream).
        # The accumulator bank is keyed by STRIP (oTs[r]): a bank is only ever
        # written by its own strip, so same-bank PE drains are strictly serial
        # (concurrent same-bank drains fault the device) while the two
        # strips/banks run concurrently.  oT = oTs[0] + oTs[1] still holds,
        # just partitioned by key-half instead of kt-parity.
        for half in range(2):
            kt = 2 * i + half
            for r in range(2):
                nc.tensor.matmul(
                    oTs[r],
                    lhsT=va[r * D : (r + 1) * D, kt, :],
                    rhs=pg[r * D : (r + 1) * D, half * QB : (half + 1) * QB],
                    start=(i == 0 and half == 0),
                    stop=(i == NPAIR - 1 and half == 1),
                    tile_position=(r * D, 0),
                )

    def epilogue(qb, oTs):
        # combine the two mm2 accumulator banks into a bf16 staging tile
        # (ACT stages bank B to SBUF since DVE reads only one PSUM operand),
        # XBAR-transpose back to [q, 65], normalize into out_sb
        tmpB = ep.tile([P, QB], F32, name="tmpB")
        nc.scalar.copy(out=tmpB[0 : D + 1, :], in_=oTs[1])
        oT_sb16 = ep.tile([P, QB], BF16, name="oT_sb16")
        # rows 65:80 are read by the padded XBAR transpose; keep them defined
        # (base partition must be 32-aligned; the add overwrites row 64)
        nc.vector.memset(oT_sb16[D : 96, :], 0.0)
        nc.vector.tensor_add(
            out=oT_sb16[0 : D + 1, :], in0=oTs[0], in1=tmpB[0 : D + 1, :]
        )
        for j in range(QB // P):
            x = 4 * qb + j
            tp16 = ep.tile([P, 80], BF16, name="tp16")
            nc.sync.dma_start(
                out=tp16, in_=oT_sb16[0:80, j * P : (j + 1) * P], transpose=True
            )
            rec = ep.tile([P, 1], F32, name="rec")
            nc.vector.reciprocal(rec, tp16[:, D : D + 1])
            nc.vector.tensor_scalar_mul(
                out_sb[:, x, :], tp16[:, 0:D], rec
            )

    # 2-group software lookahead: emit mm1(i) + exp(i), then mm2(i-2), so PE
    # always has independent mm1 work in front of it and never waits on the
    # exp of the group it is about to consume (lag 1 leaves PE stalling on
    # exp completions; measured ~16us slower).
    LAG = 2
    pend = []

    def _drain_one():
        kind, args = pend.pop(0)
        if kind == "mm2":
            emit_mm2(*args)
        else:
            epilogue(*args)

    for qb in range(NQB):
        oTs = [o_ps.tile([D + 1, QB], F32, name="oT") for _ in range(2)]
        for i in range(NPAIR):
            sg = s_ps.tile([P, 2 * QB], F32, name="sg")
            nc.tensor.matmul(
                sg[:, 0:QB],
                lhsT=kT2[0:D, i * P : (i + 1) * P],
                rhs=qT[0:D, qb * QB : (qb + 1) * QB],
                start=True,
                stop=True,
                tile_position=(0, 0),
            )
            nc.tensor.matmul(
                sg[:, QB : 2 * QB],
                lhsT=kT2[D:P, i * P : (i + 1) * P],
                rhs=qT[D:P, qb * QB : (qb + 1) * QB],
                start=True,
                stop=True,
                tile_position=(D, 0),
            )
            pg = pp.tile([P, 2 * QB], BF16, name="pg")
            if _use_act(qb, i):
                nc.scalar.activation(pg, sg, EXP)
            else:
                nc.vector.tensor_scalar(
                    out=pg.bitcast(I16),
                    in0=sg,
                    scalar1=SCH_A,
                    scalar2=SCH_B,
                    op0=mybir.AluOpType.mult,
                    op1=mybir.AluOpType.add,
                )
            pend.append(("mm2", (oTs, i, pg)))
            while len([e for e in pend if e[0] == "mm2"]) > LAG:
                _drain_one()
        pend.append(("epi", (qb, oTs)))
    while pend:
        _drain_one()

    # one flat store: out_sb[p, x, :] -> out row 32*p + x
    nc.scalar.dma_start(
        out=out.rearrange("(p x) d -> p x d", p=P), in_=out_sb
    )


def get_nc():
    if "nc" not in _CACHE:
        _CACHE["nc"] = _build()
    return _CACHE["nc"]


def kernel(q3d, k3d, v3d, _trace=False):
    q3d = np.ascontiguousarray(np.asarray(q3d, dtype=np.float32))
    k3d = np.ascontiguousarray(np.asarray(k3d, dtype=np.float32))
    v3d = np.ascontiguousarray(np.asarray(v3d, dtype=np.float32))
    assert q3d.shape == (B, S, D), q3d.shape

    nc = get_nc()
    in_maps = [{"q": q3d[b], "k": k3d[b], "v": v3d[b]} for b in range(B)]
    try:
        res = run_bass_kernel_spmd(nc, in_maps, core_ids=list(range(B)), trace=_trace)
    except Exception:
        # transient NRT/device wedges have been observed to clear on retry
        res = run_bass_kernel_spmd(nc, in_maps, core_ids=list(range(B)), trace=_trace)
    out = np.stack([res.results[b]["out"] for b in range(B)], axis=0)
    if _trace:
        return out, res
    return out


if __name__ == "__main__":
    rng = np.random.default_rng(0)
    qq = rng.standard_normal((B, S, D), dtype=np.float32)
    kk = rng.standard_normal((B, S, D), dtype=np.float32)
    vv = rng.standard_normal((B, S, D), dtype=np.float32)
    o = kernel(q3d=qq, k3d=kk, v3d=vv)
    print("kernel output:", o.shape, o.dtype)



# revision 6
# speedup vs baseline: 1.2050x; 1.0187x over previous
"""Bass/Trainium2 kernel for nn_Attention_54099408060779.

out[b] = softmax(q[b] @ k[b].T) @ v[b]   (no scaling, no mask)
B=8, S=4096, D=64, fp32 I/O.

Sharding: pure data parallel — batch b runs on NeuronCore b.

Per-core algorithm (flash-attention style, never materializes [S, S] in DRAM):
  - q, k, v are loaded as flat contiguous [128, 32, 64] f32 tiles (rows
    32*p .. 32*p+31 on partition p) so each load is a cheap 128-descriptor
    DMA, spread across the three DMA queues.  This permutes the row
    enumeration to s = 32*p + x; the permutation is carried consistently
    through the whole pipeline and undoes itself on the flat output store.
  - GPSIMD casts q,k to fp16 (dup / even-odd interleaved layouts) and v to
    bf16 (+ ones column) with strided tensor_copies — no ACT/DVE/PE cost.
  - The XBAR DMA-transpose produces qT ([dup*64+d, q], fp16, d duplicated
    in both partition halves) and kT2 ([par*64+d, key], even/odd key-tiles
    interleaved) for row-tiled mm1.
  - For each 512-wide q block, for each pair of 128-key tiles:
      sT[keys, q] = kT_pair.T @ qT_block   (TensorE, fp16, row-tiled 2x)
      p = exp(sT)  split across two engines:
        ScalarE: exact exp activation -> bf16
        VectorE: Schraudolph bit-trick -> int16 bits == bf16(exp(s)):
                 i16 = rint(s * 128*log2e + (128*127 - 5.5)), ~3.3% max err
      oT[65, q] += va.T @ p                (TensorE, bf16, ones column
                                            gives the softmax row-sums l)
  - PE-transpose oT back to [q, 65], multiply by 1/l into out_sb, and store
    the whole output with one flat DMA.

exp needs no max-subtraction: scores ~ N(0, 64), |s| < ~70 and fp32/bf16
range comfortably holds exp(70); the Schraudolph i16 stays in [3.7k, 29k].
"""

import sys

if "/opt/trn_rl_repo" not in sys.path:
    sys.path.insert(0, "/opt/trn_rl_repo")

import numpy as np

import concourse.bacc as bacc
import concourse.tile as tile
from concourse import mybir
from concourse.bass_utils import run_bass_kernel_spmd
from concourse.masks import make_identity

B, S, D = 8, 4096, 64
P = 128                # SBUF partitions / k-tile height
NKT = S // P           # 32 k-tiles (x-blocks)
NPAIR = NKT // 2       # 16 row-tiled pairs
QB = 512               # q-block width (matmul moving free dim)
NQB = S // QB          # 8 q-blocks
ROWS = S // P          # 32 rows per partition in the flat layout

BF16 = mybir.dt.bfloat16
F16 = mybir.dt.float16
F32 = mybir.dt.float32
F32R = mybir.dt.float32r
I16 = mybir.dt.int16
EXP = mybir.ActivationFunctionType.Exp

# Schraudolph constants: i16 bits of bf16(exp(s)) ~= rint(s*SCH_A + SCH_B)
SCH_A = float(128.0 * np.log2(np.e))
SCH_B = float(128.0 * 127.0 - 5.5)

# exp engine: all tiles on ScalarE (exact exp).  ACT and DVE PSUM reads
# serialize against each other on TRN2, so splitting exp across both engines
# costs the SUM of their op times; the single faster engine (ACT, 1.2 GHz)
# wins outright and frees DVE for the epilogue (A/B measured -25us vs the
# alternating split).  The Schraudolph DVE path below is kept for reference.
def _use_act(qb, i):
    return True

import os as _os
DEFAULTS = dict(
    sps=int(_os.environ.get("K_SPS", "3")),
    ops=int(_os.environ.get("K_OPS", "2")),
    eps=int(_os.environ.get("K_EPS", "8")),
)

_CACHE: dict = {}


def _build(reps: int = 1, **over):
    cfg = dict(DEFAULTS); cfg.update(over)
    nc = bacc.Bacc(None, target_bir_lowering=False)
    q = nc.dram_tensor("q", [S, D], F32, kind="ExternalInput")
    k = nc.dram_tensor("k", [S, D], F32, kind="ExternalInput")
    v = nc.dram_tensor("v", [S, D], F32, kind="ExternalInput")
    out = nc.dram_tensor("out", [S, D], F32, kind="ExternalOutput")

    with tile.TileContext(nc) as tc:
        with (
            tc.tile_pool(name="consts", bufs=1) as consts,
            tc.tile_pool(name="big", bufs=2) as big,
            tc.tile_pool(name="s_ps", bufs=cfg["sps"], space="PSUM") as s_ps,
            tc.tile_pool(name="o_ps", bufs=cfg["ops"], space="PSUM") as o_ps,
            tc.tile_pool(name="pp", bufs=8) as pp,
            tc.tile_pool(name="ep", bufs=cfg["eps"]) as ep,
        ):
            for _rep in range(reps):
                _kernel_body(nc, tc, q, k, v, out, big, s_ps, o_ps, pp, ep)

    nc.finalize()
    return nc


def _kernel_body(nc, tc, q, k, v, out, big, s_ps, o_ps, pp, ep):
    # flat f32 stages (row 32*p + x lives at [p, x, :])
    qflat = big.tile([P, ROWS, D], F32, name="qflat")
    kflat = big.tile([P, ROWS, D], F32, name="kflat")
    vflat = big.tile([P, ROWS, D], F32, name="vflat")
    # fp16 interleaved stages for the XBAR
    q16i = big.tile([P, ROWS, 2, D], F16, name="q16i")     # (x, dup, d)
    k16i = big.tile([P, NPAIR, 2, D], F16, name="k16i")    # (pair, parity, d)
    qT = big.tile([P, S], F16, name="qT")                  # [dup*64+d, x*128+p]
    kT2 = big.tile([P, NPAIR * P], F16, name="kT2")        # [par*64+d, b*128+p]
    va = big.tile([P, NKT, D + 1], BF16, name="va")
    out_sb = big.tile([P, ROWS, D], F32, name="out_sb")

    # flat loads: 2 chunks each, spread across the three DMA queues
    qsrc = q.rearrange("(p x) d -> p x d", p=P)
    ksrc = k.rearrange("(p x) d -> p x d", p=P)
    vsrc = v.rearrange("(p x) d -> p x d", p=P)
    kview = kflat.rearrange("p (b t) d -> p b t d", t=2)
    for h in range(2):
        xs = slice(h * (ROWS // 2), (h + 1) * (ROWS // 2))
        bs = slice(h * (NPAIR // 2), (h + 1) * (NPAIR // 2))
        nc.scalar.dma_start(out=qflat[:, xs, :], in_=qsrc[:, xs, :])
        nc.sync.dma_start(out=kflat[:, xs, :], in_=ksrc[:, xs, :])
        nc.gpsimd.dma_start(out=vflat[:, xs, :], in_=vsrc[:, xs, :])
        # casts + interleaves on GPSIMD (SBUF->SBUF, engine otherwise idle)
        # k16i's (pair, parity, d) order equals kflat's (x, d) order, so the
        # cast is one contiguous copy
        nc.gpsimd.tensor_copy(
            out=k16i[:, bs, :, :], in_=kview[:, bs, :, :]
        )
        for dup in range(2):
            nc.gpsimd.tensor_copy(out=q16i[:, xs, dup, :], in_=qflat[:, xs, :])
        nc.gpsimd.tensor_copy(out=va[:, xs, 0:D], in_=vflat[:, xs, :])
        # XBAR transposes (HWDGE on SP queue)
        for b in range(bs.start, bs.stop):
            nc.sync.dma_start(
                out=kT2[:, b * P : (b + 1) * P], in_=k16i[:, b, :, :], transpose=True
            )
        for x in range(xs.start, xs.stop):
            nc.sync.dma_start(
                out=qT[:, x * P : (x + 1) * P], in_=q16i[:, x, :, :], transpose=True
            )
    nc.vector.memset(va[:, :, D : D + 1], 1.0)

    def emit_mm2(oTs, i, pg):
        # Row-tiled: the K=128 key contraction splits into two K=64 strips at
        # tile_position (0,0)/(64,0) so the strips' matmuls stream through
        # disjoint array halves concurrently and each strip's LDWEIGHTS hides
        # under the other strip's in-flight matmul (the full-K version pays an
        # exposed ~230ns weight-swap per matmul, ~3x slowdown on this stream).
        # The accumulator bank is keyed by STRIP (oTs[r]): a bank is only ever
        # written by its own strip, so same-bank PE drains are strictly serial
        # (concurrent same-bank drains fault the device) while the two
        # strips/banks run concurrently.  oT = oTs[0] + oTs[1] still holds,
        # just partitioned by key-half instead of kt-parity.
        for half in range(2):
            kt = 2 * i + half
            for r in range(2):
                nc.tensor.matmul(
                    oTs[r],
                    lhsT=va[r * D : (r + 1) * D, kt, :],
                    rhs=pg[r * D : (r + 1) * D, half * QB : (half + 1) * QB],
                    start=(i == 0 and half == 0),
                    stop=(i == NPAIR - 1 and half == 1),
                    tile_position=(r * D, 0),
                )

    def epilogue(qb, oTs):
        # combine the two mm2 accumulator banks into a bf16 staging tile
        # (ACT stages bank B to SBUF since DVE reads only one PSUM operand),
        # XBAR-transpose back to [q, 65], normalize into out_sb
        tmpB = ep.tile([P, QB], F32, name="tmpB")
        nc.scalar.copy(out=tmpB[0 : D + 1, :], in_=oTs[1])
        oT_sb16 = ep.tile([P, QB], BF16, name="oT_sb16")
        # rows 65:80 are read by the padded XBAR transpose; keep them defined
        # (base partition must be 32-aligned; the add overwrites row 64)
        nc.vector.memset(oT_sb16[D : 96, :], 0.0)
        nc.vector.tensor_add(
            out=oT_sb16[0 : D + 1, :], in0=oTs[0], in1=tmpB[0 : D + 1, :]
        )
        for j in range(QB // P):
            x = 4 * qb + j
            tp16 = ep.tile([P, 80], BF16, name="tp16")
            nc.sync.dma_start(
                out=tp16, in_=oT_sb16[0:80, j * P : (j + 1) * P], transpose=True
            )
            rec = ep.tile([P, 1], F32, name="rec")
            nc.vector.reciprocal(rec, tp16[:, D : D + 1])
            nc.vector.tensor_scalar_mul(
                out_sb[:, x, :], tp16[:, 0:D], rec
            )

    # 2-group software lookahead: emit mm1(i) + exp(i), then mm2(i-2), so PE
    # always has independent mm1 work in front of it and never waits on the
    # exp of the group it is about to consume (lag 1 leaves PE stalling on
    # exp completions; measured ~16us slower).
    LAG = 2
    pend = []

    def _drain_one():
        kind, args = pend.pop(0)
        if kind == "mm2":
            emit_mm2(*args)
        else:
            epilogue(*args)

    for qb in range(NQB):
        oTs = [o_ps.tile([D + 1, QB], F32, name="oT") for _ in range(2)]
        for i in range(NPAIR):
            sg = s_ps.tile([P, 2 * QB], F32, name="sg")
            nc.tensor.matmul(
                sg[:, 0:QB],
                lhsT=kT2[0:D, i * P : (i + 1) * P],
                rhs=qT[0:D, qb * QB : (qb + 1) * QB],
                start=True,
                stop=True,
                tile_position=(0, 0),
            )
            nc.tensor.matmul(
                sg[:, QB : 2 * QB],
                lhsT=kT2[D:P, i * P : (i + 1) * P],
                rhs=qT[D:P, qb * QB : (qb + 1) * QB],
                start=True,
                stop=True,
                tile_position=(D, 0),
            )
            pg = pp.tile([P, 2 * QB], BF16, name="pg")
            if _use_act(qb, i):
                nc.scalar.activation(pg, sg, EXP)
            else:
                nc.vector.tensor_scalar(
                    out=pg.bitcast(I16),
                    in0=sg,
                    scalar1=SCH_A,
                    scalar2=SCH_B,
                    op0=mybir.AluOpType.mult,
                    op1=mybir.AluOpType.add,
                )
            pend.append(("mm2", (oTs, i, pg)))
            while len([e for e in pend if e[0] == "mm2"]) > LAG:
                _drain_one()
        pend.append(("epi", (qb, oTs)))
    while pend:
        _drain_one()

    # one flat store: out_sb[p, x, :] -> out row 32*p + x
    nc.scalar.dma_start(
        out=out.rearrange("(p x) d -> p x d", p=P), in_=out_sb
    )


def get_nc():
    if "nc" not in _CACHE:
        _CACHE["nc"] = _build()
    return _CACHE["nc"]


def kernel(q3d, k3d, v3d, _trace=False):
    q3d = np.ascontiguousarray(np.asarray(q3d, dtype=np.float32))
    k3d = np.ascontiguousarray(np.asarray(k3d, dtype=np.float32))
    v3d = np.ascontiguousarray(np.asarray(v3d, dtype=np.float32))
    assert q3d.shape == (B, S, D), q3d.shape

    nc = get_nc()
    in_maps = [{"q": q3d[b], "k": k3d[b], "v": v3d[b]} for b in range(B)]
    try:
        res = run_bass_kernel_spmd(nc, in_maps, core_ids=list(range(B)), trace=_trace)
    except Exception:
        # transient NRT/device wedges have been observed to clear on retry
        res = run_bass_kernel_spmd(nc, in_maps, core_ids=list(range(B)), trace=_trace)
    out = np.stack([res.results[b]["out"] for b in range(B)], axis=0)
    if _trace:
        return out, res
    return out


if __name__ == "__main__":
    rng = np.random.default_rng(0)
    qq = rng.standard_normal((B, S, D), dtype=np.float32)
    kk = rng.standard_normal((B, S, D), dtype=np.float32)
    vv = rng.standard_normal((B, S, D), dtype=np.float32)
    o = kernel(q3d=qq, k3d=kk, v3d=vv)
    print("kernel output:", o.shape, o.dtype)



# revision 7
# speedup vs baseline: 1.3686x; 1.1358x over previous
"""Bass/Trainium2 kernel for nn_Attention_54099408060779.

out[b] = softmax(q[b] @ k[b].T) @ v[b]   (no scaling, no mask)
B=8, S=4096, D=64, fp32 I/O.

Sharding: pure data parallel — batch b runs on NeuronCore b.

Per-core algorithm (flash-attention style, never materializes [S, S] in DRAM):
  - q, k, v are loaded as flat contiguous [128, 32, 64] f32 tiles (rows
    32*p .. 32*p+31 on partition p) so each load is a cheap 128-descriptor
    DMA, spread across the three DMA queues.  This permutes the row
    enumeration to s = 32*p + x; the permutation is carried consistently
    through the whole pipeline and undoes itself on the flat output store.
  - GPSIMD casts q,k to fp16 (dup / even-odd interleaved layouts) and v to
    bf16 (+ ones column) with strided tensor_copies — no ACT/DVE/PE cost.
  - The XBAR DMA-transpose produces qT ([dup*64+d, q], fp16, d duplicated
    in both partition halves) and kT2 ([par*64+d, key], even/odd key-tiles
    interleaved) for row-tiled mm1.
  - For each 512-wide q block, for each pair of 128-key tiles:
      sT[keys, q] = kT_pair.T @ qT_block   (TensorE, fp16, row-tiled 2x)
      p = exp(sT)  split across two engines:
        ScalarE: exact exp activation -> bf16
        VectorE: Schraudolph bit-trick -> int16 bits == bf16(exp(s)):
                 i16 = rint(s * 128*log2e + (128*127 - 5.5)), ~3.3% max err
      oT[65, q] += va.T @ p                (TensorE, bf16, ones column
                                            gives the softmax row-sums l)
  - PE-transpose oT back to [q, 65], multiply by 1/l into out_sb, and store
    the whole output with one flat DMA.

exp needs no max-subtraction: scores ~ N(0, 64), |s| < ~70 and fp32/bf16
range comfortably holds exp(70); the Schraudolph i16 stays in [3.7k, 29k].
"""

import sys

if "/opt/trn_rl_repo" not in sys.path:
    sys.path.insert(0, "/opt/trn_rl_repo")

import numpy as np

import concourse.bacc as bacc
import concourse.tile as tile
from concourse import mybir
from concourse.bass_utils import run_bass_kernel_spmd
from concourse.masks import make_identity

B, S, D = 8, 4096, 64
P = 128                # SBUF partitions / k-tile height
NKT = S // P           # 32 k-tiles (x-blocks)
NPAIR = NKT // 2       # 16 row-tiled pairs
QB = 512               # q-block width (matmul moving free dim)
NQB = S // QB          # 8 q-blocks
ROWS = S // P          # 32 rows per partition in the flat layout

BF16 = mybir.dt.bfloat16
F16 = mybir.dt.float16
F32 = mybir.dt.float32
F32R = mybir.dt.float32r
I16 = mybir.dt.int16
EXP = mybir.ActivationFunctionType.Exp

# Schraudolph constants: i16 bits of bf16(exp(s)) ~= rint(s*SCH_A + SCH_B)
SCH_A = float(128.0 * np.log2(np.e))
SCH_B = float(128.0 * 127.0 - 5.5)

# exp engine: all tiles on ScalarE (exact exp).  ACT and DVE PSUM reads
# serialize against each other on TRN2, so splitting exp across both engines
# costs the SUM of their op times; the single faster engine (ACT, 1.2 GHz)
# wins outright and frees DVE for the epilogue (A/B measured -25us vs the
# alternating split).  The Schraudolph DVE path below is kept for reference.
def _use_act(qb, i):
    return True

import os as _os
DEFAULTS = dict(
    sps=int(_os.environ.get("K_SPS", "3")),
    ops=int(_os.environ.get("K_OPS", "2")),
    eps=int(_os.environ.get("K_EPS", "8")),
)

_CACHE: dict = {}


def _build(reps: int = 1, **over):
    cfg = dict(DEFAULTS); cfg.update(over)
    nc = bacc.Bacc(None, target_bir_lowering=False)
    q = nc.dram_tensor("q", [S, D], F32, kind="ExternalInput")
    k = nc.dram_tensor("k", [S, D], F32, kind="ExternalInput")
    v = nc.dram_tensor("v", [S, D], F32, kind="ExternalInput")
    out = nc.dram_tensor("out", [S, D], F32, kind="ExternalOutput")

    with tile.TileContext(nc) as tc:
        with (
            tc.tile_pool(name="consts", bufs=1) as consts,
            tc.tile_pool(name="big", bufs=2) as big,
            tc.tile_pool(name="s_ps", bufs=cfg["sps"], space="PSUM") as s_ps,
            tc.tile_pool(name="o_ps", bufs=cfg["ops"], space="PSUM") as o_ps,
            tc.tile_pool(name="pp", bufs=8) as pp,
            tc.tile_pool(name="ep", bufs=cfg["eps"]) as ep,
        ):
            for _rep in range(reps):
                _kernel_body(nc, tc, q, k, v, out, big, s_ps, o_ps, pp, ep)

    nc.finalize()
    return nc


def _kernel_body(nc, tc, q, k, v, out, big, s_ps, o_ps, pp, ep):
    # flat f32 stages (row 32*p + x lives at [p, x, :])
    qflat = big.tile([P, ROWS, D], F32, name="qflat")
    kflat = big.tile([P, ROWS, D], F32, name="kflat")
    vflat = big.tile([P, ROWS, D], F32, name="vflat")
    # fp16 interleaved stages for the XBAR
    q16i = big.tile([P, ROWS, 2, D], F16, name="q16i")     # (x, dup, d)
    k16i = big.tile([P, NPAIR, 2, D], F16, name="k16i")    # (pair, parity, d)
    qT = big.tile([P, S], F16, name="qT")                  # [dup*64+d, x*128+p]
    kT2 = big.tile([P, NPAIR * P], F16, name="kT2")        # [par*64+d, b*128+p]
    va = big.tile([P, NKT, D + 1], BF16, name="va")
    out_sb = big.tile([P, ROWS, D], F32, name="out_sb")

    # flat loads: 2 chunks each, spread across the three DMA queues
    qsrc = q.rearrange("(p x) d -> p x d", p=P)
    ksrc = k.rearrange("(p x) d -> p x d", p=P)
    vsrc = v.rearrange("(p x) d -> p x d", p=P)
    kview = kflat.rearrange("p (b t) d -> p b t d", t=2)
    for h in range(2):
        xs = slice(h * (ROWS // 2), (h + 1) * (ROWS // 2))
        bs = slice(h * (NPAIR // 2), (h + 1) * (NPAIR // 2))
        nc.scalar.dma_start(out=qflat[:, xs, :], in_=qsrc[:, xs, :])
        nc.sync.dma_start(out=kflat[:, xs, :], in_=ksrc[:, xs, :])
        nc.gpsimd.dma_start(out=vflat[:, xs, :], in_=vsrc[:, xs, :])
        # casts + interleaves on GPSIMD (SBUF->SBUF, engine otherwise idle)
        # k16i's (pair, parity, d) order equals kflat's (x, d) order, so the
        # cast is one contiguous copy
        nc.gpsimd.tensor_copy(
            out=k16i[:, bs, :, :], in_=kview[:, bs, :, :]
        )
        for dup in range(2):
            nc.gpsimd.tensor_copy(out=q16i[:, xs, dup, :], in_=qflat[:, xs, :])
        nc.gpsimd.tensor_copy(out=va[:, xs, 0:D], in_=vflat[:, xs, :])
        # XBAR transposes (HWDGE on SP queue)
        for b in range(bs.start, bs.stop):
            nc.sync.dma_start(
                out=kT2[:, b * P : (b + 1) * P], in_=k16i[:, b, :, :], transpose=True
            )
        for x in range(xs.start, xs.stop):
            nc.sync.dma_start(
                out=qT[:, x * P : (x + 1) * P], in_=q16i[:, x, :, :], transpose=True
            )
    nc.vector.memset(va[:, :, D : D + 1], 1.0)

    def emit_mm2(oTs, i, pg):
        # Row-tiled: the K=128 key contraction splits into two K=64 strips at
        # tile_position (0,0)/(64,0) so the strips' matmuls stream through
        # disjoint array halves concurrently and each strip's LDWEIGHTS hides
        # under the other strip's in-flight matmul (the full-K version pays an
        # exposed ~230ns weight-swap per matmul, ~3x slowdown on this stream).
        # The accumulator bank is keyed by STRIP (oTs[r]): a bank is only ever
        # written by its own strip, so same-bank PE drains are strictly serial
        # (concurrent same-bank drains fault the device) while the two
        # strips/banks run concurrently.  oT = oTs[0] + oTs[1] still holds,
        # just partitioned by key-half instead of kt-parity.
        for half in range(2):
            kt = 2 * i + half
            for r in range(2):
                nc.tensor.matmul(
                    oTs[r],
                    lhsT=va[r * D : (r + 1) * D, kt, :],
                    rhs=pg[r * D : (r + 1) * D, half * QB : (half + 1) * QB],
                    start=(i == 0 and half == 0),
                    stop=(i == NPAIR - 1 and half == 1),
                    tile_position=(r * D, 0),
                )

    def epilogue(qb, oTs):
        # combine the two mm2 accumulator banks into a bf16 staging tile
        # (ACT stages bank B to SBUF since DVE reads only one PSUM operand),
        # XBAR-transpose back to [q, 65], normalize into out_sb
        tmpB = ep.tile([P, QB], F32, name="tmpB")
        # Stage bank B through DVE, not ACT: with all exp on ScalarE the DVE
        # queue is empty, so this PSUM read runs as soon as mm2 finishes
        # instead of queueing behind pending exp ops -- the oT banks release
        # sooner and the next q-block's mm2 start doesn't stall the PE.
        nc.vector.tensor_copy(out=tmpB[0 : D + 1, :], in_=oTs[1])
        oT_sb16 = ep.tile([P, QB], BF16, name="oT_sb16")
        # rows 65:80 are read by the padded XBAR transpose; keep them defined
        # (base partition must be 32-aligned; the add overwrites row 64)
        nc.vector.memset(oT_sb16[D : 96, :], 0.0)
        nc.vector.tensor_add(
            out=oT_sb16[0 : D + 1, :], in0=oTs[0], in1=tmpB[0 : D + 1, :]
        )
        for j in range(QB // P):
            x = 4 * qb + j
            tp16 = ep.tile([P, 80], BF16, name="tp16")
            nc.sync.dma_start(
                out=tp16, in_=oT_sb16[0:80, j * P : (j + 1) * P], transpose=True
            )
            rec = ep.tile([P, 1], F32, name="rec")
            nc.vector.reciprocal(rec, tp16[:, D : D + 1])
            nc.vector.tensor_scalar_mul(
                out_sb[:, x, :], tp16[:, 0:D], rec
            )

    # 2-group software lookahead: emit mm1(i) + exp(i), then mm2(i-2), so PE
    # always has independent mm1 work in front of it and never waits on the
    # exp of the group it is about to consume (lag 1 leaves PE stalling on
    # exp completions; measured ~16us slower).
    LAG = 2
    pend = []

    def _drain_one():
        kind, args = pend.pop(0)
        if kind == "mm2":
            emit_mm2(*args)
        else:
            epilogue(*args)

    for qb in range(NQB):
        oTs = [o_ps.tile([D + 1, QB], F32, name="oT") for _ in range(2)]
        for i in range(NPAIR):
            sg = s_ps.tile([P, 2 * QB], F32, name="sg")
            nc.tensor.matmul(
                sg[:, 0:QB],
                lhsT=kT2[0:D, i * P : (i + 1) * P],
                rhs=qT[0:D, qb * QB : (qb + 1) * QB],
                start=True,
                stop=True,
                tile_position=(0, 0),
            )
            nc.tensor.matmul(
                sg[:, QB : 2 * QB],
                lhsT=kT2[D:P, i * P : (i + 1) * P],
                rhs=qT[D:P, qb * QB : (qb + 1) * QB],
                start=True,
                stop=True,
                tile_position=(D, 0),
            )
            pg = pp.tile([P, 2 * QB], BF16, name="pg")
            if _use_act(qb, i):
                nc.scalar.activation(pg, sg, EXP)
            else:
                nc.vector.tensor_scalar(
                    out=pg.bitcast(I16),
                    in0=sg,
                    scalar1=SCH_A,
                    scalar2=SCH_B,
                    op0=mybir.AluOpType.mult,
                    op1=mybir.AluOpType.add,
                )
            pend.append(("mm2", (oTs, i, pg)))
            while len([e for e in pend if e[0] == "mm2"]) > LAG:
                _drain_one()
        pend.append(("epi", (qb, oTs)))
    while pend:
        _drain_one()

    # one flat store: out_sb[p, x, :] -> out row 32*p + x
    nc.scalar.dma_start(
        out=out.rearrange("(p x) d -> p x d", p=P), in_=out_sb
    )


def get_nc():
    if "nc" not in _CACHE:
        _CACHE["nc"] = _build()
    return _CACHE["nc"]


def kernel(q3d, k3d, v3d, _trace=False):
    q3d = np.ascontiguousarray(np.asarray(q3d, dtype=np.float32))
    k3d = np.ascontiguousarray(np.asarray(k3d, dtype=np.float32))
    v3d = np.ascontiguousarray(np.asarray(v3d, dtype=np.float32))
    assert q3d.shape == (B, S, D), q3d.shape

    nc = get_nc()
    in_maps = [{"q": q3d[b], "k": k3d[b], "v": v3d[b]} for b in range(B)]
    try:
        res = run_bass_kernel_spmd(nc, in_maps, core_ids=list(range(B)), trace=_trace)
    except Exception:
        # transient NRT/device wedges have been observed to clear on retry
        res = run_bass_kernel_spmd(nc, in_maps, core_ids=list(range(B)), trace=_trace)
    out = np.stack([res.results[b]["out"] for b in range(B)], axis=0)
    if _trace:
        return out, res
    return out


if __name__ == "__main__":
    rng = np.random.default_rng(0)
    qq = rng.standard_normal((B, S, D), dtype=np.float32)
    kk = rng.standard_normal((B, S, D), dtype=np.float32)
    vv = rng.standard_normal((B, S, D), dtype=np.float32)
    o = kernel(q3d=qq, k3d=kk, v3d=vv)
    print("kernel output:", o.shape, o.dtype)

